# revision 5
# baseline (speedup 1.0000x reference)
"""LoRA MLP (gate_up + SiLU*up + down, each with rank-16 LoRA) on 8 TRN2 cores.

Strategy: data-parallel over tokens (16384 = 8 x 2048); weights replicated to
every core, no collectives. All tensors bf16 (PE full rate + FWL fast weight
load + half DMA/SBUF), fp32 PSUM accumulation, fp32 output.

The rank-16 LoRA is folded into the base weights on device:
    W1' = W_gate_up + A_gate_up @ B_gate_up   (via PE matmul K=16 + DVE add)
    W2' = W_down    + A_down    @ B_down
so the steady-state loop is a pure dense MLP: clean 8-deep / 22-deep PSUM
accumulation chains with no 16-row LoRA matmuls serializing the PE.

Per core: 2 blocks of 1024 tokens. W2' (44KB/partition) stays SBUF-resident;
W1' groups are folded during block 0 and round-tripped through a DRAM scratch
for block 1 (SBUF can't hold W1'+W2'+h+x at once; write and read share one
FIFO DMA queue, which orders them). Activations stay in [feature, token]
layout so every matmul consumes natural-layout weights. Each LDWEIGHTS feeds
2 matmuls. DMA is spread over 3 queues (weights / x+consts / outputs).
PSUM: gate 2 banks + up 2 banks + one 2-bank-wide tag (fold + down-proj) x2.
"""

import numpy as np
import ml_dtypes

import concourse.mybir as mybir
import concourse.tile as tile
from concourse import bacc
from concourse.bass_utils import run_bass_kernel_spmd

TOKENS, D, FF, R = 16384, 1024, 2816, 16
N_CORES = 8
T_CORE = TOKENS // N_CORES  # 2048
BLK = 1024                  # tokens per block (2 blocks/core)
TS = 512                    # psum free-dim tile (1 bank fp32)
DT = D // 128               # 8 d-model tiles
FFT = FF // 128             # 22 ff tiles
NG = 2 * FF // 256          # 22 fold groups of 256 f-cols each
F32 = mybir.dt.float32
BF16 = mybir.dt.bfloat16
SILU = mybir.ActivationFunctionType.Silu
COPY = mybir.ActivationFunctionType.Copy
BF = ml_dtypes.bfloat16

_prog_cache = {}


def _build():
    nc = bacc.Bacc("TRN2", target_bir_lowering=False, debug=False)
    xT = nc.dram_tensor("xT", [D, T_CORE], BF16, kind="ExternalInput").ap()
    w1 = nc.dram_tensor("W_gate_up", [D, 2 * FF], BF16, kind="ExternalInput").ap()
    a1t = nc.dram_tensor("A1T", [R, D], BF16, kind="ExternalInput").ap()
    b1 = nc.dram_tensor("B_gate_up", [R, 2 * FF], BF16, kind="ExternalInput").ap()
    w2 = nc.dram_tensor("W_down", [FF, D], BF16, kind="ExternalInput").ap()
    a2t = nc.dram_tensor("A2T", [R, FF], BF16, kind="ExternalInput").ap()
    b2 = nc.dram_tensor("B_down", [R, D], BF16, kind="ExternalInput").ap()
    out = nc.dram_tensor("out", [T_CORE, D], F32, kind="ExternalOutput").ap()
    # W1' spill space so block 1 re-reads the folded weights instead of refolding
    w1s = nc.dram_tensor("w1s", [NG, 128, DT, 256], BF16, kind="Internal").ap()

    w1r = w1.rearrange("(dt p) f -> p dt f", p=128)   # [128, 8, 5632]
    w2r = w2.rearrange("(ft p) d -> p ft d", p=128)   # [128, 22, 1024]
    xTr = xT.rearrange("(dt p) t -> p dt t", p=128)   # [128, 8, 2048]

    with tile.TileContext(nc) as tc:
        with (
            tc.tile_pool(name="constp", bufs=1) as constp,
            tc.tile_pool(name="w1c", bufs=7) as w1c,      # W1' group tiles, 4KB each
            tc.tile_pool(name="w2p", bufs=1) as w2p,      # W2' resident
            tc.tile_pool(name="w1raw", bufs=3) as w1raw,
            tc.tile_pool(name="w2raw", bufs=2) as w2raw,
            tc.tile_pool(name="b1p", bufs=4) as b1p,
            tc.tile_pool(name="xp", bufs=2) as xp,
            tc.tile_pool(name="hp", bufs=1) as hp,
            tc.tile_pool(name="tmpp", bufs=4) as tmpp,
            tc.tile_pool(name="evp", bufs=2) as evp,
            tc.tile_pool(name="ps", bufs=1, space="PSUM") as ps,
        ):
            a1t_sb = constp.tile([R, D], BF16)
            nc.gpsimd.dma_start(a1t_sb[:], a1t[:])
            a2t_sb = constp.tile([R, FF], BF16)
            nc.gpsimd.dma_start(a2t_sb[:], a2t[:])
            b2_sb = constp.tile([R, D], BF16)
            nc.gpsimd.dma_start(b2_sb[:], b2[:])
            w2sb = w2p.tile([128, FFT, D], BF16)

            def fold_w1_group(g):
                """W1' cols [g*256,(g+1)*256) for all 8 d-tiles -> SBUF tile."""
                c0 = g * 256
                raw = w1raw.tile([128, DT, 256], BF16, tag="w1raw")
                nc.sync.dma_start(raw[:], w1r[:, :, c0 : c0 + 256])
                b1c = b1p.tile([R, 256], BF16, tag="b1c")
                nc.sync.dma_start(b1c[:], b1[:, c0 : c0 + 256])
                wt = w1c.tile([128, DT, 256], BF16, tag="w1c")
                for dp in range(DT // 2):
                    pf = ps.tile([128, 2, 256], F32, tag="pw", bufs=2, name="pf")
                    for k in range(2):
                        dt = 2 * dp + k
                        nc.tensor.matmul(
                            pf[:, k, :],
                            a1t_sb[:, dt * 128 : (dt + 1) * 128],
                            b1c[:],
                            start=True, stop=True,
                        )
                    nc.vector.tensor_add(
                        wt[:, 2 * dp : 2 * dp + 2, :], pf[:], raw[:, 2 * dp : 2 * dp + 2, :]
                    )
                nc.sync.dma_start(w1s[g], wt[:])
                return wt

            def load_w1_group(g):
                wt = w1c.tile([128, DT, 256], BF16, tag="w1c")
                nc.sync.dma_start(wt[:], w1s[g])
                return wt

            def fold_w2_tile(i):
                """W2' row-tile i ([128 ff, 1024 d]) into resident w2sb."""
                raw = w2raw.tile([128, D], BF16, tag="w2raw")
                nc.sync.dma_start(raw[:], w2r[:, i, :])
                pw = ps.tile([128, D], F32, tag="pw", bufs=2, name="pwf")
                for ds in range(2):
                    dsl = slice(ds * TS, (ds + 1) * TS)
                    nc.tensor.matmul(
                        pw[:, dsl],
                        a2t_sb[:, i * 128 : (i + 1) * 128],
                        b2_sb[:, dsl],
                        start=True, stop=True,
                    )
                nc.vector.tensor_add(w2sb[:, i, :], pw[:], raw[:])

            for blk in range(T_CORE // BLK):
                t0 = blk * BLK
                xt = xp.tile([128, DT, BLK], BF16, tag="xt")
                nc.gpsimd.dma_start(xt[:], xTr[:, :, t0 : t0 + BLK])
                h = hp.tile([128, FFT, BLK], BF16, tag="h")
                # ---- phase 1: h = silu(x@W1g') * (x@W1u') ----
                gtiles = {}
                for f in range(FFT):
                    g_gate, g_up = f // 2, FFT // 2 + f // 2
                    if f % 2 == 0:
                        if blk == 0:
                            gtiles[g_gate] = fold_w1_group(g_gate)
                            gtiles[g_up] = fold_w1_group(g_up)
                        else:
                            gtiles[g_gate] = load_w1_group(g_gate)
                            gtiles[g_up] = load_w1_group(g_up)
                    off = (f % 2) * 128
                    gt, ut = gtiles[g_gate], gtiles[g_up]
                    pg0 = ps.tile([128, TS], F32, tag="pg", bufs=2, name="pg0")
                    pg1 = ps.tile([128, TS], F32, tag="pg", bufs=2, name="pg1")
                    for dt in range(DT):
                        lw = gt[:, dt, off : off + 128]
                        nc.tensor.matmul(pg0[:], lw, xt[:, dt, 0:TS],
                                         start=(dt == 0), stop=(dt == DT - 1))
                        nc.tensor.matmul(pg1[:], lw, xt[:, dt, TS:BLK],
                                         start=(dt == 0), stop=(dt == DT - 1))
                    tmp0 = tmpp.tile([128, TS], BF16, tag="tmp")
                    nc.scalar.activation(tmp0[:], pg0[:], SILU)
                    tmp1 = tmpp.tile([128, TS], BF16, tag="tmp")
                    nc.scalar.activation(tmp1[:], pg1[:], SILU)
                    pu0 = ps.tile([128, TS], F32, tag="pu", bufs=2, name="pu0")
                    pu1 = ps.tile([128, TS], F32, tag="pu", bufs=2, name="pu1")
                    for dt in range(DT):
                        lw = ut[:, dt, off : off + 128]
                        nc.tensor.matmul(pu0[:], lw, xt[:, dt, 0:TS],
                                         start=(dt == 0), stop=(dt == DT - 1))
                        nc.tensor.matmul(pu1[:], lw, xt[:, dt, TS:BLK],
                                         start=(dt == 0), stop=(dt == DT - 1))
                    nc.vector.tensor_mul(h[:, f, 0:TS], tmp0[:], pu0[:])
                    nc.vector.tensor_mul(h[:, f, TS:BLK], tmp1[:], pu1[:])
                    if blk == 0:
                        fold_w2_tile(f)
                # ---- phase 2: out = h.T @ W2' ----
                for tt in range(BLK // 128):
                    ttl = slice(tt * 128, (tt + 1) * 128)
                    po = ps.tile([128, D], F32, tag="pw", bufs=2, name="po")
                    for i in range(FFT):
                        lw = h[:, i, ttl]
                        nc.tensor.matmul(po[:, 0:TS], lw, w2sb[:, i, 0:TS],
                                         start=(i == 0), stop=(i == FFT - 1))
                        nc.tensor.matmul(po[:, TS:D], lw, w2sb[:, i, TS:D],
                                         start=(i == 0), stop=(i == FFT - 1))
                    ev = evp.tile([128, D], F32, tag="ev")
                    nc.vector.tensor_copy(ev[:, 0:TS], po[:, 0:TS])
                    nc.scalar.activation(ev[:, TS:D], po[:, TS:D], COPY)
                    nc.scalar.dma_start(out[t0 + tt * 128 : t0 + (tt + 1) * 128, :], ev[:])
    nc.compile()
    return nc


def _get_prog():
    if "nc" not in _prog_cache:
        _prog_cache["nc"] = _build()
    return _prog_cache["nc"]


def run_sharded(inputs, trace=False, tmpdir=None):
    nc = _get_prog()
    x = inputs["x"]
    bf = lambda a: np.ascontiguousarray(a, dtype=BF)
    weights = {
        "W_gate_up": bf(inputs["W_gate_up"]),
        "B_gate_up": bf(inputs["B_gate_up"]),
        "A1T": bf(np.asarray(inputs["A_gate_up"]).T),
        "W_down": bf(inputs["W_down"]),
        "A2T": bf(np.asarray(inputs["A_down"]).T),
        "B_down": bf(inputs["B_down"]),
    }
    in_maps = []
    for c in range(N_CORES):
        xs = bf(np.asarray(x[c * T_CORE : (c + 1) * T_CORE]).T)
        in_maps.append({"xT": xs, **weights})
    res = run_bass_kernel_spmd(
        nc, in_maps, list(range(N_CORES)), trace=trace, tmpdir=tmpdir
    )
    outs = [res.results[c]["out"] for c in range(N_CORES)]
    full = np.concatenate(outs, axis=0)
    return full, res


def kernel(**inputs):
    full, _ = run_sharded(inputs, trace=False)
    return full
